# revision 2
# baseline (speedup 1.0000x reference)
"""GroupVQ (nn_GroupVQ_77386720740039) Trainium2 Bass kernel, 8-core data-parallel.

Pipeline per core (4 of 32 batches):
  1. Down-projection Y^T[f, w] = w_down @ z_b^T  -- z[b] in DRAM is already the
     transposed activation matrix (DIN x W), so the pre-permute is free.
  2. Per-group VQ: S = 2 x.e (fp32 PE matmuls) minus |e|^2 (host-precomputed,
     broadcast tile), argmax via DVE max/max_index (ties -> lowest index, matching
     jnp.argmin), giving code indices + per-token min-dist for the loss.
  3. Up-projection is eliminated: out_w = T_A[ov][idx_A] + T_B[ov][idx_B] where
     T_* = w_up_slice @ codebook are host-precomputed lifted tables; gathered by
     indirect DMA, summed in the DMA (accum), PE-transposed to feature-major and
     stored to the (B, C, H, W) output layout.
  4. vq_loss = COMMIT * (sum|x|^2 - sum S_max) / (N*D*G), reduced on host from
     per-core partials.
"""
import numpy as np
from contextlib import ExitStack

import concourse.bass as bass
import concourse.tile as tile
from concourse import mybir, bacc, bass_utils
from concourse.masks import make_identity

B, C, H, W = 32, 128, 12, 1200
DIN, FIX, OV, G, D, K = 1536, 768, 4, 6, 512, 1024
T = W // OV               # 300 tokens per batch
NCORES = 8
BL = B // NCORES          # 4 local batches per core
COMMIT = 0.25
P = 128

# token tiles within a batch: (t0, tlen)
TOKEN_TILES = [(0, 128), (128, 128), (256, 44)]
NTT = len(TOKEN_TILES)
GA = [0, 2, 3, 5]         # main (512-dim) group per ov phase
GB = [1, 1, 4, 4]         # shared (256-dim) group per ov phase

F32 = mybir.dt.float32
U32 = mybir.dt.uint32

_cache = {}


def _build_program():
    nc = bacc.Bacc("TRN2", target_bir_lowering=False, debug=False,
                   num_devices=NCORES)

    in_z = nc.dram_tensor("zc", (BL, DIN, W), F32, kind="ExternalInput").ap()
    in_wdt = nc.dram_tensor("wdt", (DIN, FIX), F32, kind="ExternalInput").ap()
    in_emb2 = nc.dram_tensor("emb2", (G, D, K), F32, kind="ExternalInput").ap()
    in_e2bc = nc.dram_tensor("e2bc", (G, P, K), F32, kind="ExternalInput").ap()
    in_tabs = [nc.dram_tensor(f"tab{r}{ov}", (K, DIN), F32, kind="ExternalInput").ap()
               for r in ("a", "b") for ov in range(OV)]
    tabA = in_tabs[:4]
    tabB = in_tabs[4:]

    out_z = nc.dram_tensor("outz", (BL, DIN, W), F32, kind="ExternalOutput").ap()
    out_smax = nc.dram_tensor("smax", (P, BL * G * NTT), F32, kind="ExternalOutput").ap()
    out_xsq = nc.dram_tensor("xsq", (P, BL * G), F32, kind="ExternalOutput").ap()

    KCH = DIN // P        # 12 contraction chunks for down-proj
    FCH = FIX // P        # 6 f-chunks of Y
    NW = 400              # down-proj moving chunk
    NWCH = W // NW        # 3

    with tile.TileContext(nc) as tc, ExitStack() as ctx:
        pool = ctx.enter_context(tc.tile_pool(name="sbuf", bufs=1))
        psum = ctx.enter_context(tc.tile_pool(name="psum", bufs=1, space="PSUM"))

        ident = pool.tile([P, P], F32, tag="ident")
        make_identity(nc, ident[:])

        # resident weights: w_down^T as 12 chunks [128, 768] side by side
        wdt = pool.tile([P, KCH * FIX], F32, tag="wdt")
        for kc in range(KCH):
            nc.sync.dma_start(wdt[:, kc * FIX:(kc + 1) * FIX],
                              in_wdt[kc * P:(kc + 1) * P, :])

        smax_all = pool.tile([P, BL * G * NTT], F32, tag="smax_all")
        xsq_all = pool.tile([P, BL * G], F32, tag="xsq_all")
        nc.gpsimd.memset(smax_all[:], 0.0)
        nc.gpsimd.memset(xsq_all[:], 0.0)

        for b in range(BL):
            # ---------- Phase 1: down-projection -> Y (6 tiles [128, 1200]) ----
            ytiles = [pool.tile([P, W], F32, tag=f"y{fc}", name=f"y{fc}_{b}") for fc in range(FCH)]
            for half in range(2):            # fc in {0..2} then {3..5}
                fcs = [3 * half, 3 * half + 1, 3 * half + 2]
                for nch in range(NWCH):
                    accs = {fc: psum.tile([P, NW], F32, tag=f"acc{i}",
                                           name=f"acc_{b}_{half}_{nch}_{fc}")
                            for i, fc in enumerate(fcs)}
                    for kc in range(KCH):
                        zt = pool.tile([P, NW], F32, tag="zchunk")
                        nc.sync.dma_start(
                            zt[:], in_z[b, kc * P:(kc + 1) * P,
                                        nch * NW:(nch + 1) * NW])
                        for fc in fcs:
                            nc.tensor.matmul(
                                accs[fc][:],
                                wdt[:, kc * FIX + fc * P: kc * FIX + (fc + 1) * P],
                                zt[:],
                                start=(kc == 0), stop=(kc == KCH - 1))
                    for fc in fcs:
                        nc.scalar.copy(ytiles[fc][:, nch * NW:(nch + 1) * NW],
                                       accs[fc][:])
            # |x|^2 partials for the loss (full Y tiles, ACT Square + accum)
            for fc in range(FCH):
                sq_scr = pool.tile([P, W], F32, tag="sq_scr")
                sq_col = pool.tile([P, 1], F32, tag="sq_col")
                nc.scalar.activation(out=sq_scr[:], in_=ytiles[fc][:],
                                     func=mybir.ActivationFunctionType.Square,
                                     accum_out=sq_col[:])
                nc.vector.tensor_copy(xsq_all[:, b * G + fc: b * G + fc + 1],
                                      sq_col[:])

            # ---------- Phase 2: per-group VQ -------------------------------
            idx_tiles = {}
            for g in range(G):
                er = pool.tile([P, 4 * K], F32, tag="emb2g")     # 4 k-chunks
                for kcg in range(4):
                    nc.sync.dma_start(er[:, kcg * K:(kcg + 1) * K],
                                      in_emb2[g, kcg * P:(kcg + 1) * P, :])
                e2t = pool.tile([P, K], F32, tag="e2bcg")
                nc.sync.dma_start(e2t[:], in_e2bc[g])

                for tt, (t0, tlen) in enumerate(TOKEN_TILES):
                    spsum = psum.tile([P, K], F32, tag="S")
                    for nch in range(2):                        # code chunks 512
                        for kcg in range(4):
                            gdim = 512 * g + 128 * kcg
                            ov, f0 = gdim // FIX, gdim % FIX
                            lhs = (ytiles[f0 // P][:]
                                   .rearrange("p (t o) -> p t o", o=OV)
                                   [:, t0:t0 + tlen, ov])
                            nc.tensor.matmul(
                                spsum[:tlen, nch * 512:(nch + 1) * 512],
                                lhs,
                                er[:, kcg * K + nch * 512: kcg * K + (nch + 1) * 512],
                                start=(kcg == 0), stop=(kcg == 3))
                    ssb = pool.tile([P, K], F32, tag="Ssb")
                    nc.vector.tensor_tensor(out=ssb[:tlen], in0=spsum[:tlen],
                                            in1=e2t[:tlen],
                                            op=mybir.AluOpType.subtract)
                    mx = pool.tile([P, 8], F32, tag="mx")
                    mi = pool.tile([P, 8], U32, tag="mi")
                    nc.vector.max(mx[:tlen], ssb[:tlen])
                    nc.vector.max_index(mi[:tlen], mx[:tlen], ssb[:tlen])
                    col = (b * G + g) * NTT + tt
                    nc.vector.tensor_copy(smax_all[:tlen, col:col + 1],
                                          mx[:tlen, 0:1])
                    idxt = pool.tile([P, 1], U32, tag=f"idx{g}_{tt}")
                    nc.vector.tensor_copy(idxt[:tlen], mi[:tlen, 0:1])
                    idx_tiles[(g, tt)] = idxt

            # ---------- Phase 3: gather + transpose + store -----------------
            for tt, (t0, tlen) in enumerate(TOKEN_TILES):
                obuf = pool.tile([P, FCH * 2 * 512], F32, tag="obuf")  # 12 fb x 512 w
                for ov in range(OV):
                    gsum = pool.tile([P, DIN], F32, tag="gsum")
                    nc.gpsimd.indirect_dma_start(
                        out=gsum[:tlen], out_offset=None, in_=tabA[ov],
                        in_offset=bass.IndirectOffsetOnAxis(
                            ap=idx_tiles[(GA[ov], tt)][:tlen, :1], axis=0))
                    nc.gpsimd.indirect_dma_start(
                        out=gsum[:tlen], out_offset=None, in_=tabB[ov],
                        in_offset=bass.IndirectOffsetOnAxis(
                            ap=idx_tiles[(GB[ov], tt)][:tlen, :1], axis=0),
                        compute_op=mybir.AluOpType.add)
                    for fb in range(DIN // P):                  # 12 blocks
                        trp = psum.tile([P, P], F32, tag="trp")
                        nc.tensor.transpose(
                            out=trp[:, :tlen],
                            in_=gsum[:tlen, fb * P:(fb + 1) * P],
                            identity=ident[:tlen, :tlen])
                        dst = (obuf[:, fb * 512:(fb + 1) * 512]
                               .rearrange("p (t o) -> p t o", o=OV)
                               [:, :tlen, ov])
                        nc.scalar.copy(dst, trp[:, :tlen])
                for fb in range(DIN // P):
                    nc.sync.dma_start(
                        out_z[b, fb * P:(fb + 1) * P,
                              OV * t0: OV * t0 + OV * tlen],
                        obuf[:, fb * 512: fb * 512 + OV * tlen])

        nc.sync.dma_start(out_smax, smax_all[:])
        nc.sync.dma_start(out_xsq, xsq_all[:])

    nc.compile()
    return nc


def _host_prep(z, w_down, w_up, embedding):
    z = np.ascontiguousarray(np.asarray(z, dtype=np.float32))
    w_down = np.asarray(w_down, dtype=np.float32)
    w_up = np.asarray(w_up, dtype=np.float32)
    emb = np.asarray(embedding, dtype=np.float32)

    zr = z.reshape(B, DIN, W)                      # (b, c*H+h, w) == x^T per batch
    wdt = np.ascontiguousarray(w_down.T)           # (DIN, FIX)
    emb2 = np.ascontiguousarray(2.0 * emb)         # (G, D, K)
    e2 = (emb.astype(np.float64) ** 2).sum(axis=1)  # (G, K)
    e2bc = np.ascontiguousarray(
        np.broadcast_to(e2[:, None, :], (G, P, K)).astype(np.float32))

    wu = w_up.astype(np.float64)                   # (DIN, FIX)
    em = emb.astype(np.float64)
    tabs = {}
    # main tables: full 512-dim group -> w_up column slice
    a_spec = [(0, 0, 512), (2, 256, 768), (3, 0, 512), (5, 256, 768)]
    for ov, (g, f0, f1) in enumerate(a_spec):
        tabs[f"taba{ov}"] = np.ascontiguousarray(
            (em[g].T @ wu[:, f0:f1].T).astype(np.float32))       # (K, DIN)
    # shared tables: 256-dim halves of groups 1 and 4
    b_spec = [(1, 0, 256, 512, 768), (1, 256, 512, 0, 256),
              (4, 0, 256, 512, 768), (4, 256, 512, 0, 256)]
    for ov, (g, d0, d1, f0, f1) in enumerate(b_spec):
        tabs[f"tabb{ov}"] = np.ascontiguousarray(
            (em[g, d0:d1].T @ wu[:, f0:f1].T).astype(np.float32))  # (K, DIN)

    in_maps = []
    for c in range(NCORES):
        m = {"zc": np.ascontiguousarray(zr[c * BL:(c + 1) * BL]),
             "wdt": wdt, "emb2": emb2, "e2bc": e2bc}
        m.update(tabs)
        in_maps.append(m)
    return in_maps


def kernel(z, w_down, w_up, embedding):
    if "nc" not in _cache:
        _cache["nc"] = _build_program()
    nc = _cache["nc"]

    in_maps = _host_prep(z, w_down, w_up, embedding)
    res = bass_utils.run_bass_kernel_spmd(nc, in_maps, core_ids=list(range(NCORES)))
    _cache["last_result"] = res

    zq = np.empty((B, C, H, W), dtype=np.float32)
    sum_smax = 0.0
    sum_xsq = 0.0
    for c, r in enumerate(res.results):
        zq[c * BL:(c + 1) * BL] = r["outz"].reshape(BL, C, H, W)
        sm = r["smax"].astype(np.float64)
        for col_b in range(BL):
            for g in range(G):
                for tt, (t0, tlen) in enumerate(TOKEN_TILES):
                    sum_smax += sm[:tlen, (col_b * G + g) * NTT + tt].sum()
        sum_xsq += r["xsq"].astype(np.float64).sum()

    vq_loss = np.float32(COMMIT * (sum_xsq - sum_smax) / (B * T * D * G))
    return zq, vq_loss
